# revision 45
# baseline (speedup 1.0000x reference)
"""Trainium2 Bass kernel for Transformer-XL-style relative attention.

nn module: x[1,2048,1024], 16 heads, depth 64; causal attention with
Music-Transformer skewed relative-position bias (q @ E^T + pad/reshape skew),
then output projection.

Sharding: tensor-parallel over heads, 2 heads per core on 8 cores, no
on-chip collectives. Each core computes its heads' attention and its partial
c_proj contribution; the host sums the 8 partials (the TP output reduction,
done during the un-shard gather) and adds c_proj_b.

Per-core dataflow, all in "transposed" [j, q] orientation so the attention
probabilities feed the ctx matmul directly with no transposes:
  qkvT = Wqkv^T @ x^T           bf16 matmuls, f32 PSUM accum (+ qkv bias via
                                per-partition tensor_scalar on the evacuation)
  sT[j,i] = k_j . q_i           lhsT = kT tile, rhs = qT chunk; the two heads
                                run as K=64 matmuls on disjoint PE row groups
                                into the two banks of one wide PSUM tile
  R[i,m] = q_i . E_m            written to DRAM (fp8e4m3 - E is Glorot-small
                                so the relative bias tolerates fp8) with row
                                stride S+1: the pad-trick skew. Reading the
                                same buffer with row stride S yields
                                bias[i,j] = R[i, S-1-i+j] as plain contiguous
                                "strips" [q, j] - no transposing DMA needed
  sT += strip.T                 per 128-col piece via identity-rhs matmuls:
                                the matmul itself performs the transpose
  pT = exp(sT/8)                one wide ACT pass per head pair (bounded
                                logits - no row-max needed)
  causal: only tiles with 128*jt <= i are computed; the diagonal 128-col
          block is masked with a triangular 0/1 tile after exp
  ctxT & rowsum = [v | 1]^T @ pT   fused ones-column = softmax denominator
  1/rowsum: PE broadcast (ones^T @ row) then 64-lane reciprocal_approx
  ctxTn = ctxT * (1/rowsum); head1 shifted to partitions 64-127 via an
                                identity matmul (engines cannot cross
                                partitions; the PE can)
  out_partial = ctxTn^T @ Wproj per 128-row q tile, bf16 partials to HBM
"""

import math

import numpy as np
import ml_dtypes

import concourse.bacc as bacc
import concourse.bass as bass
import concourse.mybir as mybir
from concourse import tile
from concourse.bass_utils import run_bass_kernel_spmd

BF16 = mybir.dt.bfloat16
FP8 = mybir.dt.float8e4
F32 = mybir.dt.float32
AF = mybir.ActivationFunctionType

S = 2048          # sequence length
HID = 1024        # hidden
D = 64            # head depth
T = 128           # tile edge (q rows / j cols)
CH = 512          # q-chunk width (one PSUM bank of f32)
NQC = S // CH     # 4 q-chunks
NJT = S // T      # 16 j tiles
KC = HID // T     # 8 hidden chunks
PAD = T - 1
EW = S + T        # eT padded width (2176); cols >= S are zeros
NCORES = 8
BSZ = 1048704     # skew scratch elems per (head, q-chunk)


def _m0(t):
    return S - T * (t + 1)


def _mhi(t):
    # last q tile is trimmed to m <= S so adjacent-row writes in the skew
    # buffer stay collision-free on every causally-read position
    return S + 1 if t == NJT - 1 else S + T - 1


def build(debug=False):
    nc = bacc.Bacc()
    xT = nc.declare_dram_parameter("xT", [HID, S], BF16, False)
    wqkv = nc.declare_dram_parameter("wqkv", [HID, 3 * T], BF16, False)
    wqkv_b = nc.declare_dram_parameter("wqkv_b", [T, 3], F32, False)
    eT = nc.declare_dram_parameter("eT", [T, EW], BF16, False)
    wproj = nc.declare_dram_parameter("wproj", [T, HID], BF16, False)
    tri = nc.declare_dram_parameter("tri", [T, T], BF16, False)
    iden = nc.declare_dram_parameter("iden", [T, T], BF16, False)
    iden8 = nc.declare_dram_parameter("iden8", [T, T], FP8, False)
    out = nc.declare_dram_parameter("out", [S, HID], BF16, True)
    if debug:
        d_qkv = nc.declare_dram_parameter("d_qkv", [T, 3 * S], BF16, True)
        d_v = nc.declare_dram_parameter("d_v", [T, NJT * 130], BF16, True)
        d_ctxn = nc.declare_dram_parameter("d_ctxn", [T, S], BF16, True)
        d_bias = nc.declare_dram_parameter("d_bias", [T, CH], BF16, True)
        d_pt = nc.declare_dram_parameter("d_pt", [T, CH], BF16, True)
        d_ctx = nc.declare_dram_parameter("d_ctx", [T, CH], F32, True)
        d_rs = nc.declare_dram_parameter("d_rs", [1, CH], F32, True)
        d_bc = nc.declare_dram_parameter("d_bc", [T, CH], F32, True)
    bsk = [[nc.dram_tensor(f"bsk{h}_{qc}", [BSZ], FP8) for qc in range(NQC)]
           for h in range(2)]

    with tile.TileContext(nc) as tc:
        with tc.tile_pool(name="const", bufs=1) as cp:
            xT_sb = cp.tile([T, KC, S], BF16, tag="xT")
            wq_sb = cp.tile([T, KC, 3 * T], BF16, tag="wq")
            wqb_sb = cp.tile([T, 3], F32, tag="wqb")
            eT_sb = cp.tile([T, EW], BF16, tag="eT")
            wp_sb = cp.tile([T, HID], BF16, tag="wp")
            tri_sb = cp.tile([T, T], BF16, tag="tri")
            id_sb = cp.tile([T, T], BF16, tag="iden")
            id8_sb = cp.tile([T, T], FP8, tag="iden8")
            qkvT_sb = cp.tile([T, 3, S], BF16, tag="qkvT")
            # per j-tile: [v_h0(0:64) | ones(64) | v_h1(65:129) | ones(129)]
            v_sb = cp.tile([T, NJT, 130], BF16, tag="v")
            ctxn_sb = cp.tile([T, S], BF16, tag="ctxn")
            warm_sb = cp.tile([T, 2], F32, tag="warm")
            ones_sb = cp.tile([T, 64], BF16, tag="ones")

            nc.sync.dma_start(out=wq_sb[:],
                              in_=wqkv[:].rearrange("(c p) m -> p c m", p=T))
            nc.sync.dma_start(out=wqb_sb[:], in_=wqkv_b[:])
            for kc_ in range(KC):
                nc.sync.dma_start(out=xT_sb[:, kc_, :],
                                  in_=xT[T * kc_:T * (kc_ + 1), :])
            nc.sync.dma_start(out=eT_sb[:], in_=eT[:])
            nc.sync.dma_start(out=id_sb[:], in_=iden[:])
            nc.sync.dma_start(out=id8_sb[:], in_=iden8[:])
            nc.sync.dma_start(out=wp_sb[:], in_=wproj[:])
            nc.sync.dma_start(out=tri_sb[:], in_=tri[:])

            # preload the Exp table while the big DMAs run
            nc.vector.memset(warm_sb[:, 0:1], 0.0)
            nc.scalar.activation(warm_sb[:, 1:2], warm_sb[:, 0:1], AF.Exp)

            # ones columns for the fused rowsum
            nc.vector.memset(v_sb[:, :, 64:65], 1.0)
            nc.vector.memset(v_sb[:, :, 129:130], 1.0)
            nc.vector.memset(ones_sb[:], 1.0)

            # ---------------- phase 1: qkv projection -------------------
            with tc.tile_pool(name="ps1", bufs=2, space="PSUM") as ps1:
                for nt in range(NQC):
                    for m in range(3):
                        acc = ps1.tile([T, CH], F32, tag="qkv")
                        for kc in range(KC):
                            nc.tensor.matmul(
                                acc[:],
                                wq_sb[:, kc, m * T:(m + 1) * T],
                                xT_sb[:, kc, nt * CH:(nt + 1) * CH],
                                start=(kc == 0), stop=(kc == KC - 1))
                        nc.vector.tensor_scalar_add(
                            qkvT_sb[:, m, nt * CH:(nt + 1) * CH], acc[:],
                            wqb_sb[:, m:m + 1])

            # ---------------- phase 1b: v transposes --------------------
            with tc.tile_pool(name="ps2", bufs=2, space="PSUM") as ps2:
                for jt in range(NJT):
                    pv = ps2.tile([T, T], BF16, tag="vtr")
                    nc.tensor.transpose(pv[:],
                                        qkvT_sb[:, 2, jt * T:(jt + 1) * T],
                                        id_sb[:])
                    nc.vector.tensor_copy(v_sb[:, jt, 0:64], pv[:, 0:64])
                    nc.vector.tensor_copy(v_sb[:, jt, 65:129], pv[:, 64:128])

            # ---------------- phase 2: attention ------------------------
            with (
                tc.tile_pool(name="psR", bufs=1, space="PSUM") as psR,
                tc.tile_pool(name="psS", bufs=2, space="PSUM") as psS,
                tc.tile_pool(name="psC", bufs=1, space="PSUM") as psC,
                tc.tile_pool(name="sbR", bufs=3) as sbR,
                tc.tile_pool(name="sbB", bufs=10) as sbB,
                tc.tile_pool(name="sbP", bufs=8) as sbP,
                tc.tile_pool(name="sbN", bufs=2) as sbN,
                tc.tile_pool(name="sbO", bufs=3) as sbO,
            ):
                def emit_R(qc):
                    for r in range(4):
                        t = 4 * qc + r
                        m0, mhi = _m0(t), _mhi(t)
                        W = mhi - m0
                        eRp = sbR.tile([T, 2, EW], FP8, tag="rawR")
                        for ck in range(math.ceil(W / CH)):
                            c0 = m0 + ck * CH
                            csz = min(CH, mhi - c0)
                            rp = psR.tile([T, 2 * CH], F32, tag="R")
                            for h in range(2):
                                hp = slice(64 * h, 64 * h + 64)
                                nc.tensor.matmul(
                                    rp[:, h * CH:h * CH + csz],
                                    qkvT_sb[hp, 0, t * T:(t + 1) * T],
                                    eT_sb[hp, c0:c0 + csz],
                                    start=True, stop=True,
                                    skip_group_check=True)
                            nc.any.tensor_copy(
                                eRp[:, :, ck * CH:ck * CH + csz],
                                rp[:].rearrange("p (h c) -> p h c", h=2)
                                [:, :, 0:csz])
                        for h in range(2):
                            off_w = (PAD + CH * qc + T * r * (S + 1)
                                     + m0 - (S - 1))
                            nc.sync.dma_start(
                                out=bass.AP(bsk[h][qc], off_w,
                                            [[S + 1, T], [1, W]]),
                                in_=eRp[:, h, 0:W])

                emit_R(0)
                for qc in range(NQC):
                    # --- bias strips: plain [q, j] reads, one per q tile ---
                    # (emitted before next chunk's R so the ready strip DMAs
                    # aren't stuck behind stalled skew writes in the queue)
                    strips = {}
                    for h in range(2):
                        for r in range(4):
                            qt = 4 * qc + r
                            jw = T * (qt + 1)
                            st = sbB.tile([T, S], FP8, tag="strip")
                            nc.sync.dma_start(
                                out=st[:, 0:jw],
                                in_=bass.AP(bsk[h][qc], PAD + T * r * S,
                                            [[S, T], [1, jw]]))
                            strips[(h, r)] = st
                    if qc + 1 < NQC:
                        emit_R(qc + 1)

                    # --- attention tiles ---
                    cx0 = psC.tile([T, CH], F32, tag="ctx0")
                    cx1 = psC.tile([T, CH], F32, tag="ctx1")
                    ctx_ps = [cx0, cx1]
                    jt_max = 4 * qc + 3
                    for jt in range(jt_max + 1):
                        i0 = max(CH * qc, T * jt)
                        ext = CH * (qc + 1) - i0
                        il0 = i0 - CH * qc
                        diag = (i0 == T * jt)
                        r_lo = max(jt - 4 * qc, 0)
                        spp = psS.tile([T, 2 * CH], F32, tag="sT2")
                        for h in range(2):
                            hp = slice(64 * h, 64 * h + 64)
                            nc.tensor.matmul(
                                spp[:, h * CH:h * CH + ext],
                                qkvT_sb[hp, 1, jt * T:(jt + 1) * T],
                                qkvT_sb[hp, 0, i0:i0 + ext],
                                start=True, stop=False,
                                skip_group_check=True)
                        for h in range(2):
                            # bias add: strip[:, jt-tile].T via identity rhs,
                            # one 128-col piece per q tile covered by sp
                            for r in range(r_lo, 4):
                                co = h * CH + r * T - il0
                                nc.tensor.matmul(
                                    spp[:, co:co + T],
                                    strips[(h, r)][:, jt * T:(jt + 1) * T],
                                    id8_sb[:],
                                    start=False, stop=(r == 3),
                                    skip_group_check=True)
                        pTp = sbP.tile([T, 2, CH], BF16, tag="pT")
                        nc.scalar.activation(
                            pTp[:, :, 0:ext],
                            spp[:].rearrange("p (h c) -> p h c", h=2)[:, :, 0:ext],
                            AF.Exp, scale=0.125)
                        if diag:
                            nc.vector.tensor_mul(pTp[:, 0, 0:T],
                                                 pTp[:, 0, 0:T], tri_sb[:])
                            nc.vector.tensor_mul(pTp[:, 1, 0:T],
                                                 pTp[:, 1, 0:T], tri_sb[:])
                        for h in range(2):
                            if debug and qc == 0 and jt == 0 and h == 0:
                                nc.sync.dma_start(out=d_bias[:, :ext],
                                                  in_=strips[(0, 0)][:, 0:ext])
                                nc.sync.dma_start(out=d_pt[:, :ext],
                                                  in_=pTp[:, 0, 0:ext])
                            cx = ctx_ps[h]
                            nc.tensor.matmul(
                                cx[0:65, il0:il0 + ext],
                                v_sb[:, jt, 65 * h:65 * h + 65],
                                pTp[:, h, 0:ext],
                                start=(jt == 0), stop=(jt == jt_max),
                                skip_group_check=True)

                    if debug and qc == 0:
                        dcx = sbN.tile([T, CH], F32, tag="dcx")
                        nc.vector.tensor_copy(dcx[:], ctx_ps[0][:])
                        nc.sync.dma_start(out=d_ctx[:], in_=dcx[:])
                    # --- normalize + merge heads ---
                    for h in range(2):
                        cx = ctx_ps[h]
                        # evacuate ctx PSUM early so next chunk's ctx
                        # accumulation isn't gated on the normalize chain
                        cxs = sbN.tile([T, CH], F32, tag=f"cxs{h}")
                        nc.scalar.activation(cxs[0:65, :], cx[0:65, :],
                                             AF.Copy)
                        # broadcast RAW rowsum across 64 partitions on PE
                        # (ones[1,64].T @ rowsum[1,512]), then the approx
                        # reciprocal runs 64-lane-parallel at base 0
                        rsc = sbN.tile([T, CH], BF16, tag="rsc")
                        nc.vector.tensor_copy(rsc[64:65, :], cxs[64:65, :])
                        bcpw = psS.tile([T, 2 * CH], F32, tag="sT2")
                        bcp = bcpw[:, 0:CH]
                        nc.tensor.matmul(bcp[0:64, :],
                                         ones_sb[64:65, 0:64],
                                         rsc[64:65, :],
                                         start=True, stop=True)
                        bcs = sbN.tile([T, CH], F32, tag="bcs")
                        nc.scalar.activation(bcs[0:64, :], bcp[0:64, :],
                                             AF.Copy)
                        bc = sbN.tile([T, CH], F32, tag="bc")
                        bc2 = sbN.tile([T, CH], F32, tag="bc2")
                        nc.vector.reciprocal_approx_accurate(
                            bc[0:64, :], bcs[0:64, :], bc2[0:64, :])
                        if debug and qc == 0 and h == 0:
                            nc.sync.dma_start(out=d_rs[:], in_=bc[0:1, :])
                            nc.sync.dma_start(out=d_bc[:], in_=bc[:])
                        if h == 0:
                            nc.vector.tensor_mul(
                                ctxn_sb[0:64, qc * CH:(qc + 1) * CH],
                                cxs[0:64, :], bc[0:64, :])
                        else:
                            tmp = sbN.tile([T, CH], BF16, tag="tmp1")
                            nc.vector.tensor_mul(tmp[0:64, :], cxs[0:64, :],
                                                 bc[0:64, :])
                            shw = psS.tile([T, 2 * CH], F32, tag="sT2")
                            sh = shw[:, 0:CH]
                            nc.tensor.matmul(sh[64:128, :],
                                             id_sb[0:64, 0:64], tmp[0:64, :],
                                             start=True, stop=True,
                                             tile_position=(0, 64))
                            nc.vector.tensor_copy(
                                ctxn_sb[64:128, qc * CH:(qc + 1) * CH],
                                sh[64:128, :])

                    # --- c_proj for this q-chunk ---
                    for r in range(4):
                        q0 = qc * CH + r * T
                        og = sbO.tile([T, HID], BF16, tag="outs")
                        for oc in range(2):
                            ppw = psS.tile([T, 2 * CH], F32, tag="sT2")
                            pp = ppw[:, 0:CH]
                            nc.tensor.matmul(pp[:], ctxn_sb[:, q0:q0 + T],
                                             wp_sb[:, oc * CH:(oc + 1) * CH],
                                             start=True, stop=True)
                            nc.any.tensor_copy(og[:, oc * CH:(oc + 1) * CH],
                                               pp[:])
                        nc.sync.dma_start(out=out[q0:q0 + T, :], in_=og[:])

                if debug:
                    nc.sync.dma_start(out=d_qkv[:],
                                      in_=qkvT_sb[:].rearrange("p a b -> p (a b)"))
                    nc.sync.dma_start(out=d_v[:],
                                      in_=v_sb[:].rearrange("p a b -> p (a b)"))
                    nc.sync.dma_start(out=d_ctxn[:], in_=ctxn_sb[:])

    nc.finalize()
    return nc


_NC_CACHE = {}


def _get_nc():
    if "nc" not in _NC_CACHE:
        _NC_CACHE["nc"] = build()
    return _NC_CACHE["nc"]


def _prep_core_inputs(x, c_attn_w, c_attn_b, c_proj_w, E):
    bf = ml_dtypes.bfloat16
    xT = np.ascontiguousarray(np.asarray(x)[0].T).astype(bf)     # [1024, 2048]
    c_attn_w = np.asarray(c_attn_w)
    c_attn_b = np.asarray(c_attn_b)
    c_proj_w = np.asarray(c_proj_w)
    E = np.asarray(E)
    # tri[j, q] = 1 if j <= q else 0 (upper triangular incl diagonal)
    tri = np.triu(np.ones((T, T), np.float32)).astype(bf)
    iden = np.eye(T, dtype=np.float32).astype(bf)
    iden8 = np.eye(T, dtype=np.float32).astype(ml_dtypes.float8_e4m3)
    maps = []
    for c in range(NCORES):
        qs = slice(T * c, T * (c + 1))
        wq = np.concatenate([
            c_attn_w[:, qs],
            c_attn_w[:, HID + T * c:HID + T * (c + 1)],
            c_attn_w[:, 2 * HID + T * c:2 * HID + T * (c + 1)],
        ], axis=1).astype(bf)                                    # [1024, 384]
        wqb = np.stack([
            c_attn_b[0, qs],
            c_attn_b[0, HID + T * c:HID + T * (c + 1)],
            c_attn_b[0, 2 * HID + T * c:2 * HID + T * (c + 1)],
        ], axis=1).astype(np.float32)                            # [128, 3]
        eTc = np.zeros((T, EW), np.float32)
        eTc[0:64, 0:S] = E[2 * c].T
        eTc[64:128, 0:S] = E[2 * c + 1].T
        wp = c_proj_w[T * c:T * (c + 1), :].astype(bf)           # [128, 1024]
        maps.append({
            "xT": xT, "wqkv": wq, "wqkv_b": wqb, "eT": eTc.astype(bf),
            "wproj": wp, "tri": tri, "iden": iden, "iden8": iden8,
        })
    return maps


def run_cores(inputs, trace=False, trace_kwargs=None):
    nc = _get_nc()
    maps = _prep_core_inputs(inputs["x"], inputs["c_attn_w"],
                             inputs["c_attn_b"], inputs["c_proj_w"],
                             inputs["E"])
    kw = {}
    if trace:
        kw["trace"] = True
        if trace_kwargs:
            kw.update(trace_kwargs)
    return run_bass_kernel_spmd(nc, maps, core_ids=list(range(NCORES)), **kw)


def kernel(**inputs):
    res = run_cores(inputs, trace=False)
    acc = np.zeros((S, HID), np.float32)
    for c in range(NCORES):
        acc += np.asarray(res.results[c]["out"]).astype(np.float32)
    acc += np.asarray(inputs["c_proj_b"]).astype(np.float32)
    return acc.reshape(1, S, HID)


# revision 46
# speedup vs baseline: 1.1224x; 1.1224x over previous
"""Trainium2 Bass kernel for Transformer-XL-style relative attention.

nn module: x[1,2048,1024], 16 heads, depth 64; causal attention with
Music-Transformer skewed relative-position bias (q @ E^T + pad/reshape skew),
then output projection.

Sharding: tensor-parallel over heads, 2 heads per core on 8 cores, no
on-chip collectives. Each core computes its heads' attention and its partial
c_proj contribution; the host sums the 8 partials (the TP output reduction,
done during the un-shard gather) and adds c_proj_b.

Per-core dataflow, all in "transposed" [j, q] orientation so the attention
probabilities feed the ctx matmul directly with no transposes:
  qkvT = Wqkv^T @ x^T           bf16 matmuls, f32 PSUM accum (+ qkv bias via
                                per-partition tensor_scalar on the evacuation)
  sT[j,i] = k_j . q_i           lhsT = kT tile, rhs = qT chunk; the two heads
                                run as K=64 matmuls on disjoint PE row groups
                                into the two banks of one wide PSUM tile
  R[i,m] = q_i . E_m            written to DRAM (fp8e4m3 - E is Glorot-small
                                so the relative bias tolerates fp8) with row
                                stride S+1: the pad-trick skew. Reading the
                                same buffer with row stride S yields
                                bias[i,j] = R[i, S-1-i+j] as plain contiguous
                                "strips" [q, j] - no transposing DMA needed
  sT += strip.T                 per 128-col piece via identity-rhs matmuls:
                                the matmul itself performs the transpose
  pT = exp(sT/8)                one wide ACT pass per head pair (bounded
                                logits - no row-max needed)
  causal: only tiles with 128*jt <= i are computed; the diagonal 128-col
          block is masked with a triangular 0/1 tile after exp
  ctxT & rowsum = [v | 1]^T @ pT   fused ones-column = softmax denominator
  1/rowsum: PE broadcast (ones^T @ row) then 64-lane reciprocal_approx
  ctxTn = ctxT * (1/rowsum); head1 shifted to partitions 64-127 via an
                                identity matmul (engines cannot cross
                                partitions; the PE can)
  out_partial = ctxTn^T @ Wproj per 128-row q tile, bf16 partials to HBM
"""

import math

import numpy as np
import ml_dtypes

import concourse.bacc as bacc
import concourse.bass as bass
import concourse.mybir as mybir
from concourse import tile
from concourse.bass_utils import run_bass_kernel_spmd

BF16 = mybir.dt.bfloat16
FP8 = mybir.dt.float8e4
F32 = mybir.dt.float32
AF = mybir.ActivationFunctionType

S = 2048          # sequence length
HID = 1024        # hidden
D = 64            # head depth
T = 128           # tile edge (q rows / j cols)
CH = 512          # q-chunk width (one PSUM bank of f32)
NQC = S // CH     # 4 q-chunks
NJT = S // T      # 16 j tiles
KC = HID // T     # 8 hidden chunks
PAD = T - 1
EW = S + T        # eT padded width (2176); cols >= S are zeros
NCORES = 8
BSZ = 1048704     # skew scratch elems per (head, q-chunk)


def _m0(t):
    return S - T * (t + 1)


def _mhi(t):
    # last q tile is trimmed to m <= S so adjacent-row writes in the skew
    # buffer stay collision-free on every causally-read position
    return S + 1 if t == NJT - 1 else S + T - 1


def build(debug=False):
    nc = bacc.Bacc()
    xT = nc.declare_dram_parameter("xT", [HID, S], BF16, False)
    wqkv = nc.declare_dram_parameter("wqkv", [HID, 3 * T], BF16, False)
    wqkv_b = nc.declare_dram_parameter("wqkv_b", [T, 3], F32, False)
    eT = nc.declare_dram_parameter("eT", [T, EW], BF16, False)
    wproj = nc.declare_dram_parameter("wproj", [T, HID], BF16, False)
    tri = nc.declare_dram_parameter("tri", [T, T], BF16, False)
    iden = nc.declare_dram_parameter("iden", [T, T], BF16, False)
    iden8 = nc.declare_dram_parameter("iden8", [T, T], FP8, False)
    out = nc.declare_dram_parameter("out", [S, HID], BF16, True)
    if debug:
        d_qkv = nc.declare_dram_parameter("d_qkv", [T, 3 * S], BF16, True)
        d_v = nc.declare_dram_parameter("d_v", [T, NJT * 130], BF16, True)
        d_ctxn = nc.declare_dram_parameter("d_ctxn", [T, S], BF16, True)
        d_bias = nc.declare_dram_parameter("d_bias", [T, CH], BF16, True)
        d_pt = nc.declare_dram_parameter("d_pt", [T, CH], BF16, True)
        d_ctx = nc.declare_dram_parameter("d_ctx", [T, CH], F32, True)
        d_rs = nc.declare_dram_parameter("d_rs", [1, CH], F32, True)
        d_bc = nc.declare_dram_parameter("d_bc", [T, CH], F32, True)
    bsk = [[nc.dram_tensor(f"bsk{h}_{qc}", [BSZ], FP8) for qc in range(NQC)]
           for h in range(2)]

    with tile.TileContext(nc) as tc:
        with tc.tile_pool(name="const", bufs=1) as cp:
            xT_sb = cp.tile([T, KC, S], BF16, tag="xT")
            wq_sb = cp.tile([T, KC, 3 * T], BF16, tag="wq")
            wqb_sb = cp.tile([T, 3], F32, tag="wqb")
            eT_sb = cp.tile([T, EW], BF16, tag="eT")
            wp_sb = cp.tile([T, HID], BF16, tag="wp")
            tri_sb = cp.tile([T, T], BF16, tag="tri")
            id_sb = cp.tile([T, T], BF16, tag="iden")
            id8_sb = cp.tile([T, T], FP8, tag="iden8")
            qkvT_sb = cp.tile([T, 3, S], BF16, tag="qkvT")
            # per j-tile: [v_h0(0:64) | ones(64) | v_h1(65:129) | ones(129)]
            v_sb = cp.tile([T, NJT, 130], BF16, tag="v")
            ctxn_sb = cp.tile([T, S], BF16, tag="ctxn")
            warm_sb = cp.tile([T, 2], F32, tag="warm")
            ones_sb = cp.tile([T, 64], BF16, tag="ones")

            nc.sync.dma_start(out=wq_sb[:],
                              in_=wqkv[:].rearrange("(c p) m -> p c m", p=T))
            nc.sync.dma_start(out=wqb_sb[:], in_=wqkv_b[:])
            for kc_ in range(KC):
                nc.sync.dma_start(out=xT_sb[:, kc_, :],
                                  in_=xT[T * kc_:T * (kc_ + 1), :])
            nc.sync.dma_start(out=eT_sb[:], in_=eT[:])
            nc.sync.dma_start(out=id_sb[:], in_=iden[:])
            nc.sync.dma_start(out=id8_sb[:], in_=iden8[:])
            nc.sync.dma_start(out=wp_sb[:], in_=wproj[:])
            nc.sync.dma_start(out=tri_sb[:], in_=tri[:])

            # preload the Exp table while the big DMAs run
            nc.vector.memset(warm_sb[:, 0:1], 0.0)
            nc.scalar.activation(warm_sb[:, 1:2], warm_sb[:, 0:1], AF.Exp)

            # ones columns for the fused rowsum
            nc.vector.memset(v_sb[:, :, 64:65], 1.0)
            nc.vector.memset(v_sb[:, :, 129:130], 1.0)
            nc.vector.memset(ones_sb[:], 1.0)

            # ---------------- phase 1: qkv projection -------------------
            with tc.tile_pool(name="ps1", bufs=2, space="PSUM") as ps1:
                for nt in range(NQC):
                    for m in range(3):
                        acc = ps1.tile([T, CH], F32, tag="qkv")
                        for kc in range(KC):
                            nc.tensor.matmul(
                                acc[:],
                                wq_sb[:, kc, m * T:(m + 1) * T],
                                xT_sb[:, kc, nt * CH:(nt + 1) * CH],
                                start=(kc == 0), stop=(kc == KC - 1))
                        nc.vector.tensor_scalar_add(
                            qkvT_sb[:, m, nt * CH:(nt + 1) * CH], acc[:],
                            wqb_sb[:, m:m + 1])

            # ---------------- phase 1b: v transposes --------------------
            with tc.tile_pool(name="ps2", bufs=2, space="PSUM") as ps2:
                for jt in range(NJT):
                    pv = ps2.tile([T, T], BF16, tag="vtr")
                    nc.tensor.transpose(pv[:],
                                        qkvT_sb[:, 2, jt * T:(jt + 1) * T],
                                        id_sb[:])
                    nc.vector.tensor_copy(v_sb[:, jt, 0:64], pv[:, 0:64])
                    nc.vector.tensor_copy(v_sb[:, jt, 65:129], pv[:, 64:128])

            # ---------------- phase 2: attention ------------------------
            with (
                tc.tile_pool(name="psR", bufs=1, space="PSUM") as psR,
                tc.tile_pool(name="psS", bufs=2, space="PSUM") as psS,
                tc.tile_pool(name="psC", bufs=1, space="PSUM") as psC,
                tc.tile_pool(name="sbR", bufs=3) as sbR,
                tc.tile_pool(name="sbB", bufs=12) as sbB,
                tc.tile_pool(name="sbP", bufs=8) as sbP,
                tc.tile_pool(name="sbN", bufs=3) as sbN,
                tc.tile_pool(name="sbO", bufs=3) as sbO,
            ):
                def emit_R(qc):
                    for r in range(4):
                        t = 4 * qc + r
                        m0, mhi = _m0(t), _mhi(t)
                        W = mhi - m0
                        eRp = sbR.tile([T, 2, EW], FP8, tag="rawR")
                        for ck in range(math.ceil(W / CH)):
                            c0 = m0 + ck * CH
                            csz = min(CH, mhi - c0)
                            rp = psR.tile([T, 2 * CH], F32, tag="R")
                            for h in range(2):
                                hp = slice(64 * h, 64 * h + 64)
                                nc.tensor.matmul(
                                    rp[:, h * CH:h * CH + csz],
                                    qkvT_sb[hp, 0, t * T:(t + 1) * T],
                                    eT_sb[hp, c0:c0 + csz],
                                    start=True, stop=True,
                                    skip_group_check=True)
                            nc.any.tensor_copy(
                                eRp[:, :, ck * CH:ck * CH + csz],
                                rp[:].rearrange("p (h c) -> p h c", h=2)
                                [:, :, 0:csz])
                        for h in range(2):
                            off_w = (PAD + CH * qc + T * r * (S + 1)
                                     + m0 - (S - 1))
                            nc.sync.dma_start(
                                out=bass.AP(bsk[h][qc], off_w,
                                            [[S + 1, T], [1, W]]),
                                in_=eRp[:, h, 0:W])

                emit_R(0)
                for qc in range(NQC):
                    # --- bias strips: plain [q, j] reads, one per q tile ---
                    # (emitted before next chunk's R so the ready strip DMAs
                    # aren't stuck behind stalled skew writes in the queue)
                    strips = {}
                    for h in range(2):
                        for r in range(4):
                            qt = 4 * qc + r
                            jw = T * (qt + 1)
                            st = sbB.tile([T, S], FP8, tag="strip")
                            nc.sync.dma_start(
                                out=st[:, 0:jw],
                                in_=bass.AP(bsk[h][qc], PAD + T * r * S,
                                            [[S, T], [1, jw]]))
                            strips[(h, r)] = st
                    if qc + 1 < NQC:
                        emit_R(qc + 1)

                    # --- attention tiles ---
                    cx0 = psC.tile([T, CH], F32, tag="ctx0")
                    cx1 = psC.tile([T, CH], F32, tag="ctx1")
                    ctx_ps = [cx0, cx1]
                    jt_max = 4 * qc + 3
                    for jt in range(jt_max + 1):
                        i0 = max(CH * qc, T * jt)
                        ext = CH * (qc + 1) - i0
                        il0 = i0 - CH * qc
                        diag = (i0 == T * jt)
                        r_lo = max(jt - 4 * qc, 0)
                        spp = psS.tile([T, 2 * CH], F32, tag="sT2")
                        for h in range(2):
                            hp = slice(64 * h, 64 * h + 64)
                            nc.tensor.matmul(
                                spp[:, h * CH:h * CH + ext],
                                qkvT_sb[hp, 1, jt * T:(jt + 1) * T],
                                qkvT_sb[hp, 0, i0:i0 + ext],
                                start=True, stop=False,
                                skip_group_check=True)
                        for h in range(2):
                            # bias add: strip[:, jt-tile].T via identity rhs,
                            # one 128-col piece per q tile covered by sp
                            for r in range(r_lo, 4):
                                co = h * CH + r * T - il0
                                nc.tensor.matmul(
                                    spp[:, co:co + T],
                                    strips[(h, r)][:, jt * T:(jt + 1) * T],
                                    id8_sb[:],
                                    start=False, stop=(r == 3),
                                    skip_group_check=True)
                        pTp = sbP.tile([T, 2, CH], BF16, tag="pT")
                        nc.scalar.activation(
                            pTp[:, :, 0:ext],
                            spp[:].rearrange("p (h c) -> p h c", h=2)[:, :, 0:ext],
                            AF.Exp, scale=0.125)
                        if diag:
                            nc.vector.tensor_mul(pTp[:, 0, 0:T],
                                                 pTp[:, 0, 0:T], tri_sb[:])
                            nc.vector.tensor_mul(pTp[:, 1, 0:T],
                                                 pTp[:, 1, 0:T], tri_sb[:])
                        for h in range(2):
                            if debug and qc == 0 and jt == 0 and h == 0:
                                nc.sync.dma_start(out=d_bias[:, :ext],
                                                  in_=strips[(0, 0)][:, 0:ext])
                                nc.sync.dma_start(out=d_pt[:, :ext],
                                                  in_=pTp[:, 0, 0:ext])
                            cx = ctx_ps[h]
                            nc.tensor.matmul(
                                cx[0:65, il0:il0 + ext],
                                v_sb[:, jt, 65 * h:65 * h + 65],
                                pTp[:, h, 0:ext],
                                start=(jt == 0), stop=(jt == jt_max),
                                skip_group_check=True)

                    if debug and qc == 0:
                        dcx = sbN.tile([T, CH], F32, tag="dcx")
                        nc.vector.tensor_copy(dcx[:], ctx_ps[0][:])
                        nc.sync.dma_start(out=d_ctx[:], in_=dcx[:])
                    # --- normalize + merge heads ---
                    for h in range(2):
                        cx = ctx_ps[h]
                        # evacuate ctx PSUM early so next chunk's ctx
                        # accumulation isn't gated on the normalize chain
                        cxs = sbN.tile([T, CH], F32, tag=f"cxs{h}")
                        nc.scalar.activation(cxs[0:65, :], cx[0:65, :],
                                             AF.Copy)
                        # broadcast RAW rowsum across 64 partitions on PE
                        # (ones[1,64].T @ rowsum[1,512]), then the approx
                        # reciprocal runs 64-lane-parallel at base 0
                        rsc = sbN.tile([T, CH], BF16, tag="rsc")
                        nc.vector.tensor_copy(rsc[64:65, :], cxs[64:65, :])
                        bcpw = psS.tile([T, 2 * CH], F32, tag="sT2")
                        bcp = bcpw[:, 0:CH]
                        nc.tensor.matmul(bcp[0:64, :],
                                         ones_sb[64:65, 0:64],
                                         rsc[64:65, :],
                                         start=True, stop=True)
                        bcs = sbN.tile([T, CH], F32, tag="bcs")
                        nc.scalar.activation(bcs[0:64, :], bcp[0:64, :],
                                             AF.Copy)
                        bc = sbN.tile([T, CH], F32, tag="bc")
                        bc2 = sbN.tile([T, CH], F32, tag="bc2")
                        nc.vector.reciprocal_approx_accurate(
                            bc[0:64, :], bcs[0:64, :], bc2[0:64, :])
                        if debug and qc == 0 and h == 0:
                            nc.sync.dma_start(out=d_rs[:], in_=bc[0:1, :])
                            nc.sync.dma_start(out=d_bc[:], in_=bc[:])
                        if h == 0:
                            nc.vector.tensor_mul(
                                ctxn_sb[0:64, qc * CH:(qc + 1) * CH],
                                cxs[0:64, :], bc[0:64, :])
                        else:
                            tmp = sbN.tile([T, CH], BF16, tag="tmp1")
                            nc.vector.tensor_mul(tmp[0:64, :], cxs[0:64, :],
                                                 bc[0:64, :])
                            shw = psS.tile([T, 2 * CH], F32, tag="sT2")
                            sh = shw[:, 0:CH]
                            nc.tensor.matmul(sh[64:128, :],
                                             id_sb[0:64, 0:64], tmp[0:64, :],
                                             start=True, stop=True,
                                             tile_position=(0, 64))
                            nc.vector.tensor_copy(
                                ctxn_sb[64:128, qc * CH:(qc + 1) * CH],
                                sh[64:128, :])

                    # --- c_proj for this q-chunk ---
                    for r in range(4):
                        q0 = qc * CH + r * T
                        og = sbO.tile([T, HID], BF16, tag="outs")
                        for oc in range(2):
                            ppw = psS.tile([T, 2 * CH], F32, tag="sT2")
                            pp = ppw[:, 0:CH]
                            nc.tensor.matmul(pp[:], ctxn_sb[:, q0:q0 + T],
                                             wp_sb[:, oc * CH:(oc + 1) * CH],
                                             start=True, stop=True)
                            nc.any.tensor_copy(og[:, oc * CH:(oc + 1) * CH],
                                               pp[:])
                        nc.sync.dma_start(out=out[q0:q0 + T, :], in_=og[:])

                if debug:
                    nc.sync.dma_start(out=d_qkv[:],
                                      in_=qkvT_sb[:].rearrange("p a b -> p (a b)"))
                    nc.sync.dma_start(out=d_v[:],
                                      in_=v_sb[:].rearrange("p a b -> p (a b)"))
                    nc.sync.dma_start(out=d_ctxn[:], in_=ctxn_sb[:])

    nc.finalize()
    return nc


_NC_CACHE = {}


def _get_nc():
    if "nc" not in _NC_CACHE:
        _NC_CACHE["nc"] = build()
    return _NC_CACHE["nc"]


def _prep_core_inputs(x, c_attn_w, c_attn_b, c_proj_w, E):
    bf = ml_dtypes.bfloat16
    xT = np.ascontiguousarray(np.asarray(x)[0].T).astype(bf)     # [1024, 2048]
    c_attn_w = np.asarray(c_attn_w)
    c_attn_b = np.asarray(c_attn_b)
    c_proj_w = np.asarray(c_proj_w)
    E = np.asarray(E)
    # tri[j, q] = 1 if j <= q else 0 (upper triangular incl diagonal)
    tri = np.triu(np.ones((T, T), np.float32)).astype(bf)
    iden = np.eye(T, dtype=np.float32).astype(bf)
    iden8 = np.eye(T, dtype=np.float32).astype(ml_dtypes.float8_e4m3)
    maps = []
    for c in range(NCORES):
        qs = slice(T * c, T * (c + 1))
        wq = np.concatenate([
            c_attn_w[:, qs],
            c_attn_w[:, HID + T * c:HID + T * (c + 1)],
            c_attn_w[:, 2 * HID + T * c:2 * HID + T * (c + 1)],
        ], axis=1).astype(bf)                                    # [1024, 384]
        wqb = np.stack([
            c_attn_b[0, qs],
            c_attn_b[0, HID + T * c:HID + T * (c + 1)],
            c_attn_b[0, 2 * HID + T * c:2 * HID + T * (c + 1)],
        ], axis=1).astype(np.float32)                            # [128, 3]
        eTc = np.zeros((T, EW), np.float32)
        eTc[0:64, 0:S] = E[2 * c].T
        eTc[64:128, 0:S] = E[2 * c + 1].T
        wp = c_proj_w[T * c:T * (c + 1), :].astype(bf)           # [128, 1024]
        maps.append({
            "xT": xT, "wqkv": wq, "wqkv_b": wqb, "eT": eTc.astype(bf),
            "wproj": wp, "tri": tri, "iden": iden, "iden8": iden8,
        })
    return maps


def run_cores(inputs, trace=False, trace_kwargs=None):
    nc = _get_nc()
    maps = _prep_core_inputs(inputs["x"], inputs["c_attn_w"],
                             inputs["c_attn_b"], inputs["c_proj_w"],
                             inputs["E"])
    kw = {}
    if trace:
        kw["trace"] = True
        if trace_kwargs:
            kw.update(trace_kwargs)
    return run_bass_kernel_spmd(nc, maps, core_ids=list(range(NCORES)), **kw)


def kernel(**inputs):
    res = run_cores(inputs, trace=False)
    acc = np.zeros((S, HID), np.float32)
    for c in range(NCORES):
        acc += np.asarray(res.results[c]["out"]).astype(np.float32)
    acc += np.asarray(inputs["c_proj_b"]).astype(np.float32)
    return acc.reshape(1, S, HID)
